# revision 32
# baseline (speedup 1.0000x reference)
"""Trainium2 Bass kernel for the Actor MLP (BatchNorm -> 3-layer MLP -> atan2).

Data-parallel across 8 NeuronCores: batch sharded 8192 rows/core, weights
replicated. BatchNorm batch stats via per-shard bn_stats + 8KB AllReduce.

Matmuls run in float32r (fp32 with the mantissa rounded to ~11 bits): on
TRN2 this streams at the same 1 cycle/row as bf16 but with only ~2^-12
operand rounding, which keeps the atan2 branch-cut (ty ~ 0, tx < 0) sign
flips low (~294 of 8.4M elements, rel err 1.98e-2 vs bf16's 7.7e-2).
Weights are pre-rounded to f32r granularity on the host and DMA'd directly
into f32r tiles; activations are rounded by the ACT engine on PSUM
eviction. W1/W2 don't fit SBUF in 4-byte form, so they stream from HBM per
output-column slice (host-tiled for fully-contiguous reads), overlapped
with the PE. All activations stay in transposed [feature, batch] layout so
each layer's output directly feeds the next layer's moving operand.
"""

import numpy as np

P = 128
B_CORE = 8192            # batch rows per core
BT = 512                 # batch tile (free dim of transposed activations)
NBT = B_CORE // BT       # 16
NJ = BT // P             # 4 natural [128, D_IN] tiles per batch tile
D_IN = 1024
K_IN = D_IN // P         # 8
D_H = 2048
K_H = D_H // P           # 16
D_ACT = 128
BN_EPS = 1e-5
N_CORES = 8
INV_PI = float(1.0 / np.pi)

_CACHE = {}

LAST_EXEC_NS = None
LAST_TRACE_DIR = None


def _build_nc():
    import concourse.mybir as mybir
    import concourse.tile as tile
    from concourse import bacc
    from concourse.masks import make_identity

    f32 = mybir.dt.float32
    f32r = mybir.dt.float32r
    AF = mybir.ActivationFunctionType
    ALU = mybir.AluOpType

    nc = bacc.Bacc()

    x_ext = nc.declare_dram_parameter("x", [B_CORE, D_IN], f32, isOutput=False)
    # weights pre-tiled on host: [m_slice, partition, k*128] so each m-slice
    # DMA reads one fully-contiguous block per partition
    w1t_ext = nc.declare_dram_parameter("w1t", [K_H, P, D_IN], f32r, isOutput=False)
    w2t_ext = nc.declare_dram_parameter("w2t", [K_H, P, D_H], f32r, isOutput=False)
    w3yt_ext = nc.declare_dram_parameter("w3yt", [D_H, D_ACT], f32r, isOutput=False)
    w3xt_ext = nc.declare_dram_parameter("w3xt", [D_H, D_ACT], f32r, isOutput=False)
    b1_ext = nc.declare_dram_parameter("b1r", [P, K_H], f32, isOutput=False)
    b2_ext = nc.declare_dram_parameter("b2r", [P, K_H], f32, isOutput=False)
    b3y_ext = nc.declare_dram_parameter("b3y", [P, 1], f32, isOutput=False)
    b3x_ext = nc.declare_dram_parameter("b3x", [P, 1], f32, isOutput=False)
    bnw_ext = nc.declare_dram_parameter("bnw", [P, K_IN], f32, isOutput=False)
    bnb_ext = nc.declare_dram_parameter("bnb", [P, K_IN], f32, isOutput=False)
    out_ext = nc.declare_dram_parameter("out", [B_CORE, D_ACT], f32, isOutput=True)

    with tile.TileContext(nc) as tc:
        with (
            tc.tile_pool(name="singles", bufs=1) as singles,
            tc.tile_pool(name="small", bufs=1) as small,
            tc.tile_pool(name="w1m", bufs=4) as w1_pool,
            tc.tile_pool(name="w2m", bufs=4) as w2_pool,
            tc.tile_pool(name="xnat", bufs=6) as xnat_pool,
            tc.tile_pool(name="xt", bufs=1) as xt_pool,
            tc.tile_pool(name="hid", bufs=1) as hid_pool,
            tc.tile_pool(name="epi", bufs=1) as epi_pool,
            tc.tile_pool(name="res", bufs=2) as res_pool,
            tc.tile_pool(name="xpsum", bufs=3, space="PSUM") as xpsum_pool,
            tc.tile_pool(name="mmpsum", bufs=4, space="PSUM") as mm_pool,
            tc.tile_pool(name="rpsum", bufs=1, space="PSUM") as rpsum_pool,
            tc.tile_pool(name="dram", bufs=1, space="DRAM") as dram_pool,
        ):
            # ---- constants / resident weights ----
            ident = singles.tile([P, P], f32)
            make_identity(nc, ident)

            bnws = singles.tile([P, K_IN], f32)
            nc.sync.dma_start(out=bnws, in_=bnw_ext[:])
            bnbs = singles.tile([P, K_IN], f32)
            nc.sync.dma_start(out=bnbs, in_=bnb_ext[:])

            # ---- pass 1: batch stats (transpose + bn_stats on PSUM tiles) ----
            stats = small.tile([P, K_IN, NBT, 6], f32)
            mv = small.tile([P, K_IN, 2], f32)
            # pk = [mean, E[x^2]] / N_CORES, packed for the AllReduce
            pk = small.tile([P, K_IN, 2], f32)
            for t in range(NBT):
                xn = []
                for j in range(NJ):
                    xt_tile = xnat_pool.tile([P, D_IN], f32, tag="xn")
                    r0 = t * BT + j * P
                    nc.sync.dma_start(out=xt_tile, in_=x_ext[r0 : r0 + P, :])
                    xn.append(xt_tile)
                for k in range(K_IN):
                    ps = xpsum_pool.tile([P, BT], f32, tag="xps")
                    for j in range(NJ):
                        nc.tensor.transpose(
                            ps[:, j * P : (j + 1) * P],
                            xn[j][:, k * P : (k + 1) * P],
                            ident,
                        )
                    nc.vector.bn_stats(out=stats[:, k, t, :], in_=ps)
                    if t == NBT - 1:
                        # aggregate per k as soon as its last bn_stats lands,
                        # overlapping the DVE chain with remaining transposes
                        nc.vector.bn_aggr(out=mv[:, k, :], in_=stats[:, k, :, :])

            # pass-2 constants, loaded behind the stats pass so the first X
            # tiles get the DMA queues at t=0
            w3ys = singles.tile([P, K_H, D_ACT], f32r)
            nc.sync.dma_start(out=w3ys, in_=w3yt_ext[:].rearrange("(k p) a -> p k a", p=P))
            w3xs = singles.tile([P, K_H, D_ACT], f32r)
            nc.sync.dma_start(out=w3xs, in_=w3xt_ext[:].rearrange("(k p) a -> p k a", p=P))
            b1s = singles.tile([P, K_H], f32)
            nc.sync.dma_start(out=b1s, in_=b1_ext[:])
            b2s = singles.tile([P, K_H], f32)
            nc.sync.dma_start(out=b2s, in_=b2_ext[:])
            b3ys = singles.tile([P, 1], f32)
            nc.sync.dma_start(out=b3ys, in_=b3y_ext[:])
            b3xs = singles.tile([P, 1], f32)
            nc.sync.dma_start(out=b3xs, in_=b3x_ext[:])

            # pack [mean, E[x^2]]/N_CORES for the AllReduce
            nc.vector.tensor_scalar_mul(pk[:, :, 0], mv[:, :, 0], 1.0 / N_CORES)
            nc.vector.tensor_mul(pk[:, :, 1], mv[:, :, 0], mv[:, :, 0])
            nc.vector.tensor_add(pk[:, :, 1], pk[:, :, 1], mv[:, :, 1])
            nc.vector.tensor_scalar_mul(pk[:, :, 1], pk[:, :, 1], 1.0 / N_CORES)

            cc_in = dram_pool.tile([P, K_IN, 2], f32)
            cc_out = dram_pool.tile([P, K_IN, 2], f32)
            nc.sync.dma_start(out=cc_in, in_=pk)
            nc.gpsimd.collective_compute(
                "AllReduce",
                ALU.add,
                replica_groups=[list(range(N_CORES))],
                ins=[cc_in.opt()],
                outs=[cc_out.opt()],
            )
            g = small.tile([P, K_IN, 2], f32)
            nc.sync.dma_start(out=g, in_=cc_out)

            # global mean / var -> per-feature scale & shift (1/8 pre-applied)
            gm = g[:, :, 0]
            var = small.tile([P, K_IN], f32)
            nc.vector.tensor_copy(out=var, in_=g[:, :, 1])
            gm2 = small.tile([P, K_IN], f32)
            nc.vector.tensor_mul(gm2, gm, gm)
            nc.vector.tensor_sub(var, var, gm2)
            eps_t = small.tile([P, 1], f32)
            nc.vector.memset(eps_t, BN_EPS)
            sq = small.tile([P, K_IN], f32)
            nc.scalar.activation(out=sq, in_=var, func=AF.Sqrt, bias=eps_t, scale=1.0)
            rstd = small.tile([P, K_IN], f32)
            nc.vector.reciprocal(out=rstd, in_=sq)
            scale = small.tile([P, K_IN], f32)
            nc.vector.tensor_mul(scale, bnws, rstd)
            shift = small.tile([P, K_IN], f32)
            nc.vector.tensor_mul(shift, gm, scale)
            nc.vector.tensor_sub(shift, bnbs, shift)

            # ---- pass 2: normalize + 3-layer MLP + atan2 epilogue ----
            for t in range(NBT):
                xn = []
                for j in range(NJ):
                    xt_tile = xnat_pool.tile([P, D_IN], f32, tag="xn")
                    r0 = t * BT + j * P
                    nc.sync.dma_start(out=xt_tile, in_=x_ext[r0 : r0 + P, :])
                    xn.append(xt_tile)

                # transpose + fused (x*scale + shift) normalize, round to f32r
                xt = xt_pool.tile([P, K_IN, BT], f32r)
                for k in range(K_IN):
                    ps = xpsum_pool.tile([P, BT], f32, tag="xps")
                    for j in range(NJ):
                        nc.tensor.transpose(
                            ps[:, j * P : (j + 1) * P],
                            xn[j][:, k * P : (k + 1) * P],
                            ident,
                        )
                    nc.scalar.activation(
                        out=xt[:, k, :],
                        in_=ps,
                        func=AF.Identity,
                        bias=shift[:, k : k + 1],
                        scale=scale[:, k : k + 1],
                    )

                # layer 1: h1T = relu(W1 @ xnormT + b1), W1 streamed by m-slice
                h1 = hid_pool.tile([P, K_H, BT], f32r, tag="h1")
                for m in range(K_H):
                    w1m = w1_pool.tile([P, K_IN, P], f32r, tag="w1m")
                    nc.sync.dma_start(
                        out=w1m,
                        in_=w1t_ext[m].rearrange("p (k c) -> p k c", k=K_IN),
                    )
                    acc = mm_pool.tile([P, BT], f32, tag="mm")
                    for k in range(K_IN):
                        nc.tensor.matmul(
                            acc,
                            w1m[:, k, :],
                            xt[:, k, :],
                            start=(k == 0),
                            stop=(k == K_IN - 1),
                        )
                    nc.scalar.activation(
                        out=h1[:, m, :],
                        in_=acc,
                        func=AF.Relu,
                        bias=b1s[:, m : m + 1],
                        scale=1.0,
                    )

                # layer 2: h2T = relu(W2 @ h1T + b2), W2 streamed by m-slice
                h2 = hid_pool.tile([P, K_H, BT], f32r, tag="h2")
                for m in range(K_H):
                    w2m = w2_pool.tile([P, K_H, P], f32r, tag="w2m")
                    nc.sync.dma_start(
                        out=w2m,
                        in_=w2t_ext[m].rearrange("p (k c) -> p k c", k=K_H),
                    )
                    acc = mm_pool.tile([P, BT], f32, tag="mm")
                    for k in range(K_H):
                        nc.tensor.matmul(
                            acc,
                            w2m[:, k, :],
                            h1[:, k, :],
                            start=(k == 0),
                            stop=(k == K_H - 1),
                        )
                    nc.scalar.activation(
                        out=h2[:, m, :],
                        in_=acc,
                        func=AF.Relu,
                        bias=b2s[:, m : m + 1],
                        scale=1.0,
                    )

                # layer 3: tyT/txT = tanh(W3{y,x} @ h2T + b3{y,x})
                ty = epi_pool.tile([P, BT], f32, tag="ty")
                tx = epi_pool.tile([P, BT], f32, tag="tx")
                for ws, bs, dst in ((w3ys, b3ys, ty), (w3xs, b3xs, tx)):
                    acc = mm_pool.tile([P, BT], f32, tag="mm")
                    for k in range(K_H):
                        nc.tensor.matmul(
                            acc,
                            ws[:, k, :],
                            h2[:, k, :],
                            start=(k == 0),
                            stop=(k == K_H - 1),
                        )
                    nc.scalar.activation(out=dst, in_=acc, func=AF.Tanh, bias=bs, scale=1.0)

                # atan2(ty, tx)/pi = Arctan(ty/tx)/pi + sign(ty)*(1-sign(tx))/2
                rx = epi_pool.tile([P, BT], f32, tag="rx")
                nc.vector.reciprocal(out=rx, in_=tx)
                q = epi_pool.tile([P, BT], f32, tag="q")
                nc.vector.tensor_mul(q, ty, rx)
                a = epi_pool.tile([P, BT], f32, tag="a")
                nc.scalar.activation(out=a, in_=q, func=AF.Arctan)
                sy = epi_pool.tile([P, BT], f32, tag="sy")
                nc.scalar.activation(out=sy, in_=ty, func=AF.Sign)
                sx = epi_pool.tile([P, BT], f32, tag="sx")
                nc.scalar.activation(out=sx, in_=tx, func=AF.Sign)
                d = epi_pool.tile([P, BT], f32, tag="d")
                nc.vector.tensor_mul(d, sy, sx)
                nc.vector.tensor_sub(d, sy, d)
                rT = epi_pool.tile([P, BT], f32, tag="rT")
                nc.vector.tensor_scalar(
                    out=rT, in0=a, scalar1=INV_PI, scalar2=None, op0=ALU.mult
                )
                nc.vector.tensor_scalar(
                    out=d, in0=d, scalar1=0.5, scalar2=None, op0=ALU.mult
                )
                nc.vector.tensor_add(rT, rT, d)

                # transpose result back to [batch, action] and store
                for j in range(NJ):
                    rp = rpsum_pool.tile([P, P], f32, tag="rps")
                    nc.tensor.transpose(rp, rT[:, j * P : (j + 1) * P], ident)
                    rn = res_pool.tile([P, P], f32, tag="rn")
                    nc.scalar.activation(out=rn, in_=rp, func=AF.Copy)
                    r0 = t * BT + j * P
                    nc.sync.dma_start(out=out_ext[r0 : r0 + P, :], in_=rn)

    return nc


def _round_f32r(a):
    """Round-to-nearest to f32r granularity (low 12 mantissa bits zeroed)."""
    a = np.ascontiguousarray(np.asarray(a, np.float32))
    b = a.view(np.uint32)
    b = ((b + 0x800) & np.uint32(0xFFFFF000)).astype(np.uint32)
    return b.view(np.float32)


def _tile_w(w, k_tiles):
    """[M, K] row-major -> [M/128, 128p, K] where [m, p, k*128+c] = w[m*128+c, k*128+p]."""
    m_tiles = w.shape[0] // P
    t = w.reshape(m_tiles, P, k_tiles, P).transpose(0, 3, 2, 1)
    return np.ascontiguousarray(t.reshape(m_tiles, P, k_tiles * P))


def _host_prep(states, bn_weight, bn_bias, w1, b1, w2, b2, w3, b3):
    w1t = _round_f32r(_tile_w(np.asarray(w1, np.float32), K_IN))
    w2t = _round_f32r(_tile_w(np.asarray(w2, np.float32), K_H))
    w3 = np.asarray(w3, np.float32)
    w3yt = _round_f32r(w3[0::2].T)   # [D_H, D_ACT]
    w3xt = _round_f32r(w3[1::2].T)
    b1r = np.ascontiguousarray(np.asarray(b1, np.float32).reshape(K_H, P).T)
    b2r = np.ascontiguousarray(np.asarray(b2, np.float32).reshape(K_H, P).T)
    b3 = np.asarray(b3, np.float32)
    b3y = np.ascontiguousarray(b3[0::2].reshape(P, 1))
    b3x = np.ascontiguousarray(b3[1::2].reshape(P, 1))
    bnw = np.ascontiguousarray(np.asarray(bn_weight, np.float32).reshape(K_IN, P).T)
    bnb = np.ascontiguousarray(np.asarray(bn_bias, np.float32).reshape(K_IN, P).T)
    shared = {
        "w1t": w1t, "w2t": w2t, "w3yt": w3yt, "w3xt": w3xt,
        "b1r": b1r, "b2r": b2r, "b3y": b3y, "b3x": b3x,
        "bnw": bnw, "bnb": bnb,
    }
    states = np.asarray(states, np.float32)
    in_maps = []
    for c in range(N_CORES):
        m = dict(shared)
        m["x"] = np.ascontiguousarray(states[c * B_CORE : (c + 1) * B_CORE])
        in_maps.append(m)
    return in_maps


def _get_ntff_hook():
    """Best-effort NTFF profiling hook (axon images without antenv.axon_hooks)."""
    try:
        from antenv.axon_hooks import get_axon_ntff_profile_hook

        return get_axon_ntff_profile_hook()
    except ImportError:
        pass
    try:
        from trn_agent_boot.trn_boot import _ntff_profile_via_ctypes

        return _ntff_profile_via_ctypes("/opt/axon/libaxon_pjrt.so")
    except Exception:
        return None


def _run(nc, in_maps, profile=True):
    """Run the SPMD kernel via PJRT; return (per-core results, exec_time_ns)."""
    import glob
    import os
    import tempfile

    from concourse import bass2jax

    hook = _get_ntff_hook() if profile else None
    if hook is None:
        return bass2jax.run_bass_via_pjrt(nc, in_maps, n_cores=N_CORES), None, None

    tmpdir = tempfile.mkdtemp(prefix="bass_ntff_")
    try:
        with hook(tmpdir, [0]):
            results = bass2jax.run_bass_via_pjrt(nc, in_maps, n_cores=N_CORES)
    except Exception as e:
        print(f"[kernel] NTFF hook failed ({type(e).__name__}: {e}); plain run")
        return bass2jax.run_bass_via_pjrt(nc, in_maps, n_cores=N_CORES), None, None

    exec_ns = None
    try:
        if glob.glob(os.path.join(tmpdir, "*_body*.ntff")):
            import gauge.profiler
            from concourse._compat import FishPath

            profile_obj = gauge.profiler.Profile(
                profile_path=FishPath(tmpdir),
                kernel_dev_mode=True,
                profile_on_exit=False,
                bass_kernel=nc.m,
                offline_processing=True,
                fname="*_body*",
            )
            prs = profile_obj.to_perfetto(model_index=(0,))
            if prs:
                exec_ns = max(p.exec_time_ns for p in prs if p.exec_time_ns)
    except Exception as e:
        print(f"[kernel] NTFF parse failed ({type(e).__name__}: {e})")
    return results, exec_ns, tmpdir


def kernel(**inputs):
    global LAST_EXEC_NS, LAST_TRACE_DIR
    if "nc" not in _CACHE:
        nc = _build_nc()
        if not nc.is_finalized():
            nc.finalize()
        _CACHE["nc"] = nc
    nc = _CACHE["nc"]

    in_maps = _host_prep(**inputs)
    results, exec_ns, trace_dir = _run(nc, in_maps)
    LAST_EXEC_NS = exec_ns
    LAST_TRACE_DIR = trace_dir
    out = np.concatenate([results[c]["out"] for c in range(N_CORES)], axis=0)
    return out.astype(np.float32)


# revision 33
# speedup vs baseline: 1.0299x; 1.0299x over previous
"""Trainium2 Bass kernel for the Actor MLP (BatchNorm -> 3-layer MLP -> atan2).

Data-parallel across 8 NeuronCores: batch sharded 8192 rows/core, weights
replicated. BatchNorm batch stats via per-shard bn_stats + 8KB AllReduce.

Matmuls run in float32r (fp32 with the mantissa rounded to ~11 bits): on
TRN2 this streams at the same 1 cycle/row as bf16 but with only ~2^-12
operand rounding, which keeps the atan2 branch-cut (ty ~ 0, tx < 0) sign
flips low (~294 of 8.4M elements, rel err 1.98e-2 vs bf16's 7.7e-2).
Weights are pre-rounded to f32r granularity on the host and DMA'd directly
into f32r tiles; activations are rounded by the ACT engine on PSUM
eviction. W1/W2 don't fit SBUF in 4-byte form, so they stream from HBM per
output-column slice (host-tiled for fully-contiguous reads), overlapped
with the PE. All activations stay in transposed [feature, batch] layout so
each layer's output directly feeds the next layer's moving operand.
"""

import numpy as np

P = 128
B_CORE = 8192            # batch rows per core
BT = 512                 # batch tile (free dim of transposed activations)
NBT = B_CORE // BT       # 16
NJ = BT // P             # 4 natural [128, D_IN] tiles per batch tile
D_IN = 1024
K_IN = D_IN // P         # 8
D_H = 2048
K_H = D_H // P           # 16
D_ACT = 128
BN_EPS = 1e-5
N_CORES = 8
INV_PI = float(1.0 / np.pi)

_CACHE = {}

LAST_EXEC_NS = None
LAST_TRACE_DIR = None


def _build_nc():
    import concourse.mybir as mybir
    import concourse.tile as tile
    from concourse import bacc
    from concourse.masks import make_identity

    f32 = mybir.dt.float32
    f32r = mybir.dt.float32r
    AF = mybir.ActivationFunctionType
    ALU = mybir.AluOpType

    nc = bacc.Bacc()

    x_ext = nc.declare_dram_parameter("x", [B_CORE, D_IN], f32, isOutput=False)
    # weights pre-tiled on host: [m_slice, partition, k*128] so each m-slice
    # DMA reads one fully-contiguous block per partition
    w1t_ext = nc.declare_dram_parameter("w1t", [K_H, P, D_IN], f32r, isOutput=False)
    w2t_ext = nc.declare_dram_parameter("w2t", [K_H, P, D_H], f32r, isOutput=False)
    w3yt_ext = nc.declare_dram_parameter("w3yt", [D_H, D_ACT], f32r, isOutput=False)
    w3xt_ext = nc.declare_dram_parameter("w3xt", [D_H, D_ACT], f32r, isOutput=False)
    b1_ext = nc.declare_dram_parameter("b1r", [P, K_H], f32, isOutput=False)
    b2_ext = nc.declare_dram_parameter("b2r", [P, K_H], f32, isOutput=False)
    b3y_ext = nc.declare_dram_parameter("b3y", [P, 1], f32, isOutput=False)
    b3x_ext = nc.declare_dram_parameter("b3x", [P, 1], f32, isOutput=False)
    bnw_ext = nc.declare_dram_parameter("bnw", [P, K_IN], f32, isOutput=False)
    bnb_ext = nc.declare_dram_parameter("bnb", [P, K_IN], f32, isOutput=False)
    out_ext = nc.declare_dram_parameter("out", [B_CORE, D_ACT], f32, isOutput=True)

    with tile.TileContext(nc) as tc:
        with (
            tc.tile_pool(name="singles", bufs=1) as singles,
            tc.tile_pool(name="small", bufs=1) as small,
            tc.tile_pool(name="w1m", bufs=4) as w1_pool,
            tc.tile_pool(name="w2m", bufs=4) as w2_pool,
            tc.tile_pool(name="xnat", bufs=8) as xnat_pool,
            tc.tile_pool(name="xt", bufs=1) as xt_pool,
            tc.tile_pool(name="hid", bufs=1) as hid_pool,
            tc.tile_pool(name="epi", bufs=1) as epi_pool,
            tc.tile_pool(name="res", bufs=2) as res_pool,
            tc.tile_pool(name="xpsum", bufs=3, space="PSUM") as xpsum_pool,
            tc.tile_pool(name="mmpsum", bufs=4, space="PSUM") as mm_pool,
            tc.tile_pool(name="rpsum", bufs=1, space="PSUM") as rpsum_pool,
            tc.tile_pool(name="dram", bufs=1, space="DRAM") as dram_pool,
        ):
            # ---- constants / resident weights ----
            ident = singles.tile([P, P], f32)
            make_identity(nc, ident)

            bnws = singles.tile([P, K_IN], f32)
            nc.sync.dma_start(out=bnws, in_=bnw_ext[:])
            bnbs = singles.tile([P, K_IN], f32)
            nc.sync.dma_start(out=bnbs, in_=bnb_ext[:])

            # ---- pass 1: batch stats (transpose + bn_stats on PSUM tiles) ----
            stats = small.tile([P, K_IN, NBT, 6], f32)
            mv = small.tile([P, K_IN, 2], f32)
            # pk = [mean, E[x^2]] / N_CORES, packed for the AllReduce
            pk = small.tile([P, K_IN, 2], f32)
            for t in range(NBT):
                xn = []
                for j in range(NJ):
                    xt_tile = xnat_pool.tile([P, D_IN], f32, tag="xn")
                    r0 = t * BT + j * P
                    nc.sync.dma_start(out=xt_tile, in_=x_ext[r0 : r0 + P, :])
                    xn.append(xt_tile)
                for k in range(K_IN):
                    ps = xpsum_pool.tile([P, BT], f32, tag="xps")
                    for j in range(NJ):
                        nc.tensor.transpose(
                            ps[:, j * P : (j + 1) * P],
                            xn[j][:, k * P : (k + 1) * P],
                            ident,
                        )
                    nc.vector.bn_stats(out=stats[:, k, t, :], in_=ps)
                    if t == NBT - 1:
                        # aggregate per k as soon as its last bn_stats lands,
                        # overlapping the DVE chain with remaining transposes
                        nc.vector.bn_aggr(out=mv[:, k, :], in_=stats[:, k, :, :])

            # pass-2 constants, loaded behind the stats pass so the first X
            # tiles get the DMA queues at t=0
            w3ys = singles.tile([P, K_H, D_ACT], f32r)
            nc.sync.dma_start(out=w3ys, in_=w3yt_ext[:].rearrange("(k p) a -> p k a", p=P))
            w3xs = singles.tile([P, K_H, D_ACT], f32r)
            nc.sync.dma_start(out=w3xs, in_=w3xt_ext[:].rearrange("(k p) a -> p k a", p=P))
            b1s = singles.tile([P, K_H], f32)
            nc.sync.dma_start(out=b1s, in_=b1_ext[:])
            b2s = singles.tile([P, K_H], f32)
            nc.sync.dma_start(out=b2s, in_=b2_ext[:])
            b3ys = singles.tile([P, 1], f32)
            nc.sync.dma_start(out=b3ys, in_=b3y_ext[:])
            b3xs = singles.tile([P, 1], f32)
            nc.sync.dma_start(out=b3xs, in_=b3x_ext[:])

            # pack [mean, E[x^2]]/N_CORES for the AllReduce
            nc.vector.tensor_scalar_mul(pk[:, :, 0], mv[:, :, 0], 1.0 / N_CORES)
            nc.vector.tensor_mul(pk[:, :, 1], mv[:, :, 0], mv[:, :, 0])
            nc.vector.tensor_add(pk[:, :, 1], pk[:, :, 1], mv[:, :, 1])
            nc.vector.tensor_scalar_mul(pk[:, :, 1], pk[:, :, 1], 1.0 / N_CORES)

            cc_in = dram_pool.tile([P, K_IN, 2], f32)
            cc_out = dram_pool.tile([P, K_IN, 2], f32)
            nc.sync.dma_start(out=cc_in, in_=pk)
            nc.gpsimd.collective_compute(
                "AllReduce",
                ALU.add,
                replica_groups=[list(range(N_CORES))],
                ins=[cc_in.opt()],
                outs=[cc_out.opt()],
            )
            g = small.tile([P, K_IN, 2], f32)
            nc.sync.dma_start(out=g, in_=cc_out)

            # global mean / var -> per-feature scale & shift (1/8 pre-applied)
            gm = g[:, :, 0]
            var = small.tile([P, K_IN], f32)
            nc.vector.tensor_copy(out=var, in_=g[:, :, 1])
            gm2 = small.tile([P, K_IN], f32)
            nc.vector.tensor_mul(gm2, gm, gm)
            nc.vector.tensor_sub(var, var, gm2)
            eps_t = small.tile([P, 1], f32)
            nc.vector.memset(eps_t, BN_EPS)
            sq = small.tile([P, K_IN], f32)
            nc.scalar.activation(out=sq, in_=var, func=AF.Sqrt, bias=eps_t, scale=1.0)
            rstd = small.tile([P, K_IN], f32)
            nc.vector.reciprocal(out=rstd, in_=sq)
            scale = small.tile([P, K_IN], f32)
            nc.vector.tensor_mul(scale, bnws, rstd)
            shift = small.tile([P, K_IN], f32)
            nc.vector.tensor_mul(shift, gm, scale)
            nc.vector.tensor_sub(shift, bnbs, shift)

            # ---- pass 2: normalize + 3-layer MLP + atan2 epilogue ----
            for t in range(NBT):
                xn = []
                for j in range(NJ):
                    xt_tile = xnat_pool.tile([P, D_IN], f32, tag="xn")
                    r0 = t * BT + j * P
                    nc.sync.dma_start(out=xt_tile, in_=x_ext[r0 : r0 + P, :])
                    xn.append(xt_tile)

                # transpose + fused (x*scale + shift) normalize, round to f32r
                xt = xt_pool.tile([P, K_IN, BT], f32r)
                for k in range(K_IN):
                    ps = xpsum_pool.tile([P, BT], f32, tag="xps")
                    for j in range(NJ):
                        nc.tensor.transpose(
                            ps[:, j * P : (j + 1) * P],
                            xn[j][:, k * P : (k + 1) * P],
                            ident,
                        )
                    nc.scalar.activation(
                        out=xt[:, k, :],
                        in_=ps,
                        func=AF.Identity,
                        bias=shift[:, k : k + 1],
                        scale=scale[:, k : k + 1],
                    )

                # layer 1: h1T = relu(W1 @ xnormT + b1), W1 streamed by m-slice
                h1 = hid_pool.tile([P, K_H, BT], f32r, tag="h1")
                for m in range(K_H):
                    w1m = w1_pool.tile([P, K_IN, P], f32r, tag="w1m")
                    nc.sync.dma_start(
                        out=w1m,
                        in_=w1t_ext[m].rearrange("p (k c) -> p k c", k=K_IN),
                    )
                    acc = mm_pool.tile([P, BT], f32, tag="mm")
                    for k in range(K_IN):
                        nc.tensor.matmul(
                            acc,
                            w1m[:, k, :],
                            xt[:, k, :],
                            start=(k == 0),
                            stop=(k == K_IN - 1),
                        )
                    nc.scalar.activation(
                        out=h1[:, m, :],
                        in_=acc,
                        func=AF.Relu,
                        bias=b1s[:, m : m + 1],
                        scale=1.0,
                    )

                # layer 2: h2T = relu(W2 @ h1T + b2), W2 streamed by m-slice
                h2 = hid_pool.tile([P, K_H, BT], f32r, tag="h2")
                for m in range(K_H):
                    w2m = w2_pool.tile([P, K_H, P], f32r, tag="w2m")
                    nc.sync.dma_start(
                        out=w2m,
                        in_=w2t_ext[m].rearrange("p (k c) -> p k c", k=K_H),
                    )
                    acc = mm_pool.tile([P, BT], f32, tag="mm")
                    for k in range(K_H):
                        nc.tensor.matmul(
                            acc,
                            w2m[:, k, :],
                            h1[:, k, :],
                            start=(k == 0),
                            stop=(k == K_H - 1),
                        )
                    nc.scalar.activation(
                        out=h2[:, m, :],
                        in_=acc,
                        func=AF.Relu,
                        bias=b2s[:, m : m + 1],
                        scale=1.0,
                    )

                # layer 3: tyT/txT = tanh(W3{y,x} @ h2T + b3{y,x})
                ty = epi_pool.tile([P, BT], f32, tag="ty")
                tx = epi_pool.tile([P, BT], f32, tag="tx")
                for ws, bs, dst in ((w3ys, b3ys, ty), (w3xs, b3xs, tx)):
                    acc = mm_pool.tile([P, BT], f32, tag="mm")
                    for k in range(K_H):
                        nc.tensor.matmul(
                            acc,
                            ws[:, k, :],
                            h2[:, k, :],
                            start=(k == 0),
                            stop=(k == K_H - 1),
                        )
                    nc.scalar.activation(out=dst, in_=acc, func=AF.Tanh, bias=bs, scale=1.0)

                # atan2(ty, tx)/pi = Arctan(ty/tx)/pi + sign(ty)*(1-sign(tx))/2
                # (tiles reused in place to keep the epi pool at 5 tags)
                rx = epi_pool.tile([P, BT], f32, tag="rx")
                nc.vector.reciprocal(out=rx, in_=tx)
                nc.vector.tensor_mul(rx, ty, rx)            # q = ty/tx
                sy = epi_pool.tile([P, BT], f32, tag="sy")
                nc.scalar.activation(out=sy, in_=ty, func=AF.Sign)
                sx = epi_pool.tile([P, BT], f32, tag="sx")
                nc.scalar.activation(out=sx, in_=tx, func=AF.Sign)
                nc.scalar.activation(out=tx, in_=rx, func=AF.Arctan)  # a (tx dead)
                nc.vector.tensor_mul(sx, sy, sx)            # sy*sx
                nc.vector.tensor_sub(sy, sy, sx)            # d = sy*(1-sx)
                nc.vector.tensor_scalar(
                    out=rx, in0=tx, scalar1=INV_PI, scalar2=None, op0=ALU.mult
                )
                nc.vector.tensor_scalar(
                    out=sy, in0=sy, scalar1=0.5, scalar2=None, op0=ALU.mult
                )
                nc.vector.tensor_add(rx, rx, sy)            # resT

                # transpose result back to [batch, action] and store
                for j in range(NJ):
                    rp = rpsum_pool.tile([P, P], f32, tag="rps")
                    nc.tensor.transpose(rp, rx[:, j * P : (j + 1) * P], ident)
                    rn = res_pool.tile([P, P], f32, tag="rn")
                    nc.scalar.activation(out=rn, in_=rp, func=AF.Copy)
                    r0 = t * BT + j * P
                    nc.sync.dma_start(out=out_ext[r0 : r0 + P, :], in_=rn)

    return nc


def _round_f32r(a):
    """Round-to-nearest to f32r granularity (low 12 mantissa bits zeroed)."""
    a = np.ascontiguousarray(np.asarray(a, np.float32))
    b = a.view(np.uint32)
    b = ((b + 0x800) & np.uint32(0xFFFFF000)).astype(np.uint32)
    return b.view(np.float32)


def _tile_w(w, k_tiles):
    """[M, K] row-major -> [M/128, 128p, K] where [m, p, k*128+c] = w[m*128+c, k*128+p]."""
    m_tiles = w.shape[0] // P
    t = w.reshape(m_tiles, P, k_tiles, P).transpose(0, 3, 2, 1)
    return np.ascontiguousarray(t.reshape(m_tiles, P, k_tiles * P))


def _host_prep(states, bn_weight, bn_bias, w1, b1, w2, b2, w3, b3):
    w1t = _round_f32r(_tile_w(np.asarray(w1, np.float32), K_IN))
    w2t = _round_f32r(_tile_w(np.asarray(w2, np.float32), K_H))
    w3 = np.asarray(w3, np.float32)
    w3yt = _round_f32r(w3[0::2].T)   # [D_H, D_ACT]
    w3xt = _round_f32r(w3[1::2].T)
    b1r = np.ascontiguousarray(np.asarray(b1, np.float32).reshape(K_H, P).T)
    b2r = np.ascontiguousarray(np.asarray(b2, np.float32).reshape(K_H, P).T)
    b3 = np.asarray(b3, np.float32)
    b3y = np.ascontiguousarray(b3[0::2].reshape(P, 1))
    b3x = np.ascontiguousarray(b3[1::2].reshape(P, 1))
    bnw = np.ascontiguousarray(np.asarray(bn_weight, np.float32).reshape(K_IN, P).T)
    bnb = np.ascontiguousarray(np.asarray(bn_bias, np.float32).reshape(K_IN, P).T)
    shared = {
        "w1t": w1t, "w2t": w2t, "w3yt": w3yt, "w3xt": w3xt,
        "b1r": b1r, "b2r": b2r, "b3y": b3y, "b3x": b3x,
        "bnw": bnw, "bnb": bnb,
    }
    states = np.asarray(states, np.float32)
    in_maps = []
    for c in range(N_CORES):
        m = dict(shared)
        m["x"] = np.ascontiguousarray(states[c * B_CORE : (c + 1) * B_CORE])
        in_maps.append(m)
    return in_maps


def _get_ntff_hook():
    """Best-effort NTFF profiling hook (axon images without antenv.axon_hooks)."""
    try:
        from antenv.axon_hooks import get_axon_ntff_profile_hook

        return get_axon_ntff_profile_hook()
    except ImportError:
        pass
    try:
        from trn_agent_boot.trn_boot import _ntff_profile_via_ctypes

        return _ntff_profile_via_ctypes("/opt/axon/libaxon_pjrt.so")
    except Exception:
        return None


def _run(nc, in_maps, profile=True):
    """Run the SPMD kernel via PJRT; return (per-core results, exec_time_ns)."""
    import glob
    import os
    import tempfile

    from concourse import bass2jax

    hook = _get_ntff_hook() if profile else None
    if hook is None:
        return bass2jax.run_bass_via_pjrt(nc, in_maps, n_cores=N_CORES), None, None

    tmpdir = tempfile.mkdtemp(prefix="bass_ntff_")
    try:
        with hook(tmpdir, [0]):
            results = bass2jax.run_bass_via_pjrt(nc, in_maps, n_cores=N_CORES)
    except Exception as e:
        print(f"[kernel] NTFF hook failed ({type(e).__name__}: {e}); plain run")
        return bass2jax.run_bass_via_pjrt(nc, in_maps, n_cores=N_CORES), None, None

    exec_ns = None
    try:
        if glob.glob(os.path.join(tmpdir, "*_body*.ntff")):
            import gauge.profiler
            from concourse._compat import FishPath

            profile_obj = gauge.profiler.Profile(
                profile_path=FishPath(tmpdir),
                kernel_dev_mode=True,
                profile_on_exit=False,
                bass_kernel=nc.m,
                offline_processing=True,
                fname="*_body*",
            )
            prs = profile_obj.to_perfetto(model_index=(0,))
            if prs:
                exec_ns = max(p.exec_time_ns for p in prs if p.exec_time_ns)
    except Exception as e:
        print(f"[kernel] NTFF parse failed ({type(e).__name__}: {e})")
    return results, exec_ns, tmpdir


def kernel(**inputs):
    global LAST_EXEC_NS, LAST_TRACE_DIR
    if "nc" not in _CACHE:
        nc = _build_nc()
        if not nc.is_finalized():
            nc.finalize()
        _CACHE["nc"] = nc
    nc = _CACHE["nc"]

    in_maps = _host_prep(**inputs)
    results, exec_ns, trace_dir = _run(nc, in_maps)
    LAST_EXEC_NS = exec_ns
    LAST_TRACE_DIR = trace_dir
    out = np.concatenate([results[c]["out"] for c in range(N_CORES)], axis=0)
    return out.astype(np.float32)


# revision 34
# speedup vs baseline: 1.0302x; 1.0003x over previous
"""Trainium2 Bass kernel for the Actor MLP (BatchNorm -> 3-layer MLP -> atan2).

Data-parallel across 8 NeuronCores: batch sharded 8192 rows/core, weights
replicated. BatchNorm batch stats via per-shard bn_stats + 8KB AllReduce.

Matmuls run in float32r (fp32 with the mantissa rounded to ~11 bits): on
TRN2 this streams at the same 1 cycle/row as bf16 but with only ~2^-12
operand rounding, which keeps the atan2 branch-cut (ty ~ 0, tx < 0) sign
flips low (~294 of 8.4M elements, rel err 1.98e-2 vs bf16's 7.7e-2).
Weights are pre-rounded to f32r granularity on the host and DMA'd directly
into f32r tiles; activations are rounded by the ACT engine on PSUM
eviction. W1/W2 don't fit SBUF in 4-byte form, so they stream from HBM per
output-column slice (host-tiled for fully-contiguous reads), overlapped
with the PE. All activations stay in transposed [feature, batch] layout so
each layer's output directly feeds the next layer's moving operand.
"""

import numpy as np

P = 128
B_CORE = 8192            # batch rows per core
BT = 512                 # batch tile (free dim of transposed activations)
NBT = B_CORE // BT       # 16
NJ = BT // P             # 4 natural [128, D_IN] tiles per batch tile
D_IN = 1024
K_IN = D_IN // P         # 8
D_H = 2048
K_H = D_H // P           # 16
D_ACT = 128
BN_EPS = 1e-5
N_CORES = 8
INV_PI = float(1.0 / np.pi)

_CACHE = {}

LAST_EXEC_NS = None
LAST_TRACE_DIR = None


def _build_nc():
    import concourse.mybir as mybir
    import concourse.tile as tile
    from concourse import bacc
    from concourse.masks import make_identity

    f32 = mybir.dt.float32
    f32r = mybir.dt.float32r
    AF = mybir.ActivationFunctionType
    ALU = mybir.AluOpType

    nc = bacc.Bacc()

    x_ext = nc.declare_dram_parameter("x", [B_CORE, D_IN], f32, isOutput=False)
    # weights pre-tiled on host: [m_slice, partition, k*128] so each m-slice
    # DMA reads one fully-contiguous block per partition
    w1t_ext = nc.declare_dram_parameter("w1t", [K_H, P, D_IN], f32r, isOutput=False)
    w2t_ext = nc.declare_dram_parameter("w2t", [K_H, P, D_H], f32r, isOutput=False)
    w3yt_ext = nc.declare_dram_parameter("w3yt", [D_H, D_ACT], f32r, isOutput=False)
    w3xt_ext = nc.declare_dram_parameter("w3xt", [D_H, D_ACT], f32r, isOutput=False)
    b1_ext = nc.declare_dram_parameter("b1r", [P, K_H], f32, isOutput=False)
    b2_ext = nc.declare_dram_parameter("b2r", [P, K_H], f32, isOutput=False)
    b3y_ext = nc.declare_dram_parameter("b3y", [P, 1], f32, isOutput=False)
    b3x_ext = nc.declare_dram_parameter("b3x", [P, 1], f32, isOutput=False)
    bnw_ext = nc.declare_dram_parameter("bnw", [P, K_IN], f32, isOutput=False)
    bnb_ext = nc.declare_dram_parameter("bnb", [P, K_IN], f32, isOutput=False)
    out_ext = nc.declare_dram_parameter("out", [B_CORE, D_ACT], f32, isOutput=True)

    with tile.TileContext(nc) as tc:
        with (
            tc.tile_pool(name="singles", bufs=1) as singles,
            tc.tile_pool(name="small", bufs=1) as small,
            tc.tile_pool(name="w1m", bufs=4) as w1_pool,
            tc.tile_pool(name="w2m", bufs=4) as w2_pool,
            tc.tile_pool(name="xnat", bufs=8) as xnat_pool,
            tc.tile_pool(name="xt", bufs=1) as xt_pool,
            tc.tile_pool(name="hid", bufs=1) as hid_pool,
            tc.tile_pool(name="epi", bufs=1) as epi_pool,
            tc.tile_pool(name="res", bufs=2) as res_pool,
            tc.tile_pool(name="xpsum", bufs=3, space="PSUM") as xpsum_pool,
            tc.tile_pool(name="mmpsum", bufs=4, space="PSUM") as mm_pool,
            tc.tile_pool(name="rpsum", bufs=1, space="PSUM") as rpsum_pool,
            tc.tile_pool(name="dram", bufs=1, space="DRAM") as dram_pool,
        ):
            # ---- constants / resident weights ----
            ident = singles.tile([P, P], f32)
            make_identity(nc, ident)

            bnws = singles.tile([P, K_IN], f32)
            nc.sync.dma_start(out=bnws, in_=bnw_ext[:])
            bnbs = singles.tile([P, K_IN], f32)
            nc.sync.dma_start(out=bnbs, in_=bnb_ext[:])

            # ---- pass 1: batch stats (transpose + bn_stats on PSUM tiles) ----
            stats = small.tile([P, K_IN, NBT, 6], f32)
            mv = small.tile([P, K_IN, 2], f32)
            # pk = [mean, E[x^2]] / N_CORES, packed for the AllReduce
            pk = small.tile([P, K_IN, 2], f32)
            for t in range(NBT):
                xn = []
                for j in range(NJ):
                    xt_tile = xnat_pool.tile([P, D_IN], f32, tag="xn")
                    r0 = t * BT + j * P
                    nc.sync.dma_start(out=xt_tile, in_=x_ext[r0 : r0 + P, :])
                    xn.append(xt_tile)
                for k in range(K_IN):
                    ps = xpsum_pool.tile([P, BT], f32, tag="xps")
                    for j in range(NJ):
                        nc.tensor.transpose(
                            ps[:, j * P : (j + 1) * P],
                            xn[j][:, k * P : (k + 1) * P],
                            ident,
                        )
                    nc.vector.bn_stats(out=stats[:, k, t, :], in_=ps)
                    if t == NBT - 1:
                        # aggregate per k as soon as its last bn_stats lands,
                        # overlapping the DVE chain with remaining transposes
                        nc.vector.bn_aggr(out=mv[:, k, :], in_=stats[:, k, :, :])

            # pass-2 constants, loaded behind the stats pass so the first X
            # tiles get the DMA queues at t=0
            w3ys = singles.tile([P, K_H, D_ACT], f32r)
            nc.sync.dma_start(out=w3ys, in_=w3yt_ext[:].rearrange("(k p) a -> p k a", p=P))
            w3xs = singles.tile([P, K_H, D_ACT], f32r)
            nc.sync.dma_start(out=w3xs, in_=w3xt_ext[:].rearrange("(k p) a -> p k a", p=P))
            b1s = singles.tile([P, K_H], f32)
            nc.sync.dma_start(out=b1s, in_=b1_ext[:])
            b2s = singles.tile([P, K_H], f32)
            nc.sync.dma_start(out=b2s, in_=b2_ext[:])
            b3ys = singles.tile([P, 1], f32)
            nc.sync.dma_start(out=b3ys, in_=b3y_ext[:])
            b3xs = singles.tile([P, 1], f32)
            nc.sync.dma_start(out=b3xs, in_=b3x_ext[:])

            # pack [mean, E[x^2]]/N_CORES for the AllReduce
            nc.vector.tensor_scalar_mul(pk[:, :, 0], mv[:, :, 0], 1.0 / N_CORES)
            nc.vector.tensor_mul(pk[:, :, 1], mv[:, :, 0], mv[:, :, 0])
            nc.vector.tensor_add(pk[:, :, 1], pk[:, :, 1], mv[:, :, 1])
            nc.vector.tensor_scalar_mul(pk[:, :, 1], pk[:, :, 1], 1.0 / N_CORES)

            cc_in = dram_pool.tile([P, K_IN, 2], f32)
            cc_out = dram_pool.tile([P, K_IN, 2], f32)
            # gpsimd's queue, so this 8KB doesn't sit behind W2 prefetch MBs
            nc.gpsimd.dma_start(out=cc_in, in_=pk)
            nc.gpsimd.collective_compute(
                "AllReduce",
                ALU.add,
                replica_groups=[list(range(N_CORES))],
                ins=[cc_in.opt()],
                outs=[cc_out.opt()],
            )
            g = small.tile([P, K_IN, 2], f32)
            nc.gpsimd.dma_start(out=g, in_=cc_out)

            # global mean / var -> per-feature scale & shift (1/8 pre-applied)
            gm = g[:, :, 0]
            var = small.tile([P, K_IN], f32)
            nc.vector.tensor_copy(out=var, in_=g[:, :, 1])
            gm2 = small.tile([P, K_IN], f32)
            nc.vector.tensor_mul(gm2, gm, gm)
            nc.vector.tensor_sub(var, var, gm2)
            eps_t = small.tile([P, 1], f32)
            nc.vector.memset(eps_t, BN_EPS)
            sq = small.tile([P, K_IN], f32)
            nc.scalar.activation(out=sq, in_=var, func=AF.Sqrt, bias=eps_t, scale=1.0)
            rstd = small.tile([P, K_IN], f32)
            nc.vector.reciprocal(out=rstd, in_=sq)
            scale = small.tile([P, K_IN], f32)
            nc.vector.tensor_mul(scale, bnws, rstd)
            shift = small.tile([P, K_IN], f32)
            nc.vector.tensor_mul(shift, gm, scale)
            nc.vector.tensor_sub(shift, bnbs, shift)

            # ---- pass 2: normalize + 3-layer MLP + atan2 epilogue ----
            for t in range(NBT):
                xn = []
                for j in range(NJ):
                    xt_tile = xnat_pool.tile([P, D_IN], f32, tag="xn")
                    r0 = t * BT + j * P
                    nc.sync.dma_start(out=xt_tile, in_=x_ext[r0 : r0 + P, :])
                    xn.append(xt_tile)

                # transpose + fused (x*scale + shift) normalize, round to f32r
                xt = xt_pool.tile([P, K_IN, BT], f32r)
                for k in range(K_IN):
                    ps = xpsum_pool.tile([P, BT], f32, tag="xps")
                    for j in range(NJ):
                        nc.tensor.transpose(
                            ps[:, j * P : (j + 1) * P],
                            xn[j][:, k * P : (k + 1) * P],
                            ident,
                        )
                    nc.scalar.activation(
                        out=xt[:, k, :],
                        in_=ps,
                        func=AF.Identity,
                        bias=shift[:, k : k + 1],
                        scale=scale[:, k : k + 1],
                    )

                # layer 1: h1T = relu(W1 @ xnormT + b1), W1 streamed by m-slice
                h1 = hid_pool.tile([P, K_H, BT], f32r, tag="h1")
                for m in range(K_H):
                    w1m = w1_pool.tile([P, K_IN, P], f32r, tag="w1m")
                    nc.sync.dma_start(
                        out=w1m,
                        in_=w1t_ext[m].rearrange("p (k c) -> p k c", k=K_IN),
                    )
                    acc = mm_pool.tile([P, BT], f32, tag="mm")
                    for k in range(K_IN):
                        nc.tensor.matmul(
                            acc,
                            w1m[:, k, :],
                            xt[:, k, :],
                            start=(k == 0),
                            stop=(k == K_IN - 1),
                        )
                    nc.scalar.activation(
                        out=h1[:, m, :],
                        in_=acc,
                        func=AF.Relu,
                        bias=b1s[:, m : m + 1],
                        scale=1.0,
                    )

                # layer 2: h2T = relu(W2 @ h1T + b2), W2 streamed by m-slice
                h2 = hid_pool.tile([P, K_H, BT], f32r, tag="h2")
                for m in range(K_H):
                    w2m = w2_pool.tile([P, K_H, P], f32r, tag="w2m")
                    nc.sync.dma_start(
                        out=w2m,
                        in_=w2t_ext[m].rearrange("p (k c) -> p k c", k=K_H),
                    )
                    acc = mm_pool.tile([P, BT], f32, tag="mm")
                    for k in range(K_H):
                        nc.tensor.matmul(
                            acc,
                            w2m[:, k, :],
                            h1[:, k, :],
                            start=(k == 0),
                            stop=(k == K_H - 1),
                        )
                    nc.scalar.activation(
                        out=h2[:, m, :],
                        in_=acc,
                        func=AF.Relu,
                        bias=b2s[:, m : m + 1],
                        scale=1.0,
                    )

                # layer 3: tyT/txT = tanh(W3{y,x} @ h2T + b3{y,x})
                ty = epi_pool.tile([P, BT], f32, tag="ty")
                tx = epi_pool.tile([P, BT], f32, tag="tx")
                for ws, bs, dst in ((w3ys, b3ys, ty), (w3xs, b3xs, tx)):
                    acc = mm_pool.tile([P, BT], f32, tag="mm")
                    for k in range(K_H):
                        nc.tensor.matmul(
                            acc,
                            ws[:, k, :],
                            h2[:, k, :],
                            start=(k == 0),
                            stop=(k == K_H - 1),
                        )
                    nc.scalar.activation(out=dst, in_=acc, func=AF.Tanh, bias=bs, scale=1.0)

                # atan2(ty, tx)/pi = Arctan(ty/tx)/pi + sign(ty)*(1-sign(tx))/2
                # (tiles reused in place to keep the epi pool at 5 tags)
                rx = epi_pool.tile([P, BT], f32, tag="rx")
                nc.vector.reciprocal(out=rx, in_=tx)
                nc.vector.tensor_mul(rx, ty, rx)            # q = ty/tx
                sy = epi_pool.tile([P, BT], f32, tag="sy")
                nc.scalar.activation(out=sy, in_=ty, func=AF.Sign)
                sx = epi_pool.tile([P, BT], f32, tag="sx")
                nc.scalar.activation(out=sx, in_=tx, func=AF.Sign)
                nc.scalar.activation(out=tx, in_=rx, func=AF.Arctan)  # a (tx dead)
                nc.vector.tensor_mul(sx, sy, sx)            # sy*sx
                nc.vector.tensor_sub(sy, sy, sx)            # d = sy*(1-sx)
                nc.vector.tensor_scalar(
                    out=rx, in0=tx, scalar1=INV_PI, scalar2=None, op0=ALU.mult
                )
                nc.vector.tensor_scalar(
                    out=sy, in0=sy, scalar1=0.5, scalar2=None, op0=ALU.mult
                )
                nc.vector.tensor_add(rx, rx, sy)            # resT

                # transpose result back to [batch, action] and store
                for j in range(NJ):
                    rp = rpsum_pool.tile([P, P], f32, tag="rps")
                    nc.tensor.transpose(rp, rx[:, j * P : (j + 1) * P], ident)
                    rn = res_pool.tile([P, P], f32, tag="rn")
                    nc.scalar.activation(out=rn, in_=rp, func=AF.Copy)
                    r0 = t * BT + j * P
                    nc.sync.dma_start(out=out_ext[r0 : r0 + P, :], in_=rn)

    return nc


def _round_f32r(a):
    """Round-to-nearest to f32r granularity (low 12 mantissa bits zeroed)."""
    a = np.ascontiguousarray(np.asarray(a, np.float32))
    b = a.view(np.uint32)
    b = ((b + 0x800) & np.uint32(0xFFFFF000)).astype(np.uint32)
    return b.view(np.float32)


def _tile_w(w, k_tiles):
    """[M, K] row-major -> [M/128, 128p, K] where [m, p, k*128+c] = w[m*128+c, k*128+p]."""
    m_tiles = w.shape[0] // P
    t = w.reshape(m_tiles, P, k_tiles, P).transpose(0, 3, 2, 1)
    return np.ascontiguousarray(t.reshape(m_tiles, P, k_tiles * P))


def _host_prep(states, bn_weight, bn_bias, w1, b1, w2, b2, w3, b3):
    w1t = _round_f32r(_tile_w(np.asarray(w1, np.float32), K_IN))
    w2t = _round_f32r(_tile_w(np.asarray(w2, np.float32), K_H))
    w3 = np.asarray(w3, np.float32)
    w3yt = _round_f32r(w3[0::2].T)   # [D_H, D_ACT]
    w3xt = _round_f32r(w3[1::2].T)
    b1r = np.ascontiguousarray(np.asarray(b1, np.float32).reshape(K_H, P).T)
    b2r = np.ascontiguousarray(np.asarray(b2, np.float32).reshape(K_H, P).T)
    b3 = np.asarray(b3, np.float32)
    b3y = np.ascontiguousarray(b3[0::2].reshape(P, 1))
    b3x = np.ascontiguousarray(b3[1::2].reshape(P, 1))
    bnw = np.ascontiguousarray(np.asarray(bn_weight, np.float32).reshape(K_IN, P).T)
    bnb = np.ascontiguousarray(np.asarray(bn_bias, np.float32).reshape(K_IN, P).T)
    shared = {
        "w1t": w1t, "w2t": w2t, "w3yt": w3yt, "w3xt": w3xt,
        "b1r": b1r, "b2r": b2r, "b3y": b3y, "b3x": b3x,
        "bnw": bnw, "bnb": bnb,
    }
    states = np.asarray(states, np.float32)
    in_maps = []
    for c in range(N_CORES):
        m = dict(shared)
        m["x"] = np.ascontiguousarray(states[c * B_CORE : (c + 1) * B_CORE])
        in_maps.append(m)
    return in_maps


def _get_ntff_hook():
    """Best-effort NTFF profiling hook (axon images without antenv.axon_hooks)."""
    try:
        from antenv.axon_hooks import get_axon_ntff_profile_hook

        return get_axon_ntff_profile_hook()
    except ImportError:
        pass
    try:
        from trn_agent_boot.trn_boot import _ntff_profile_via_ctypes

        return _ntff_profile_via_ctypes("/opt/axon/libaxon_pjrt.so")
    except Exception:
        return None


def _run(nc, in_maps, profile=True):
    """Run the SPMD kernel via PJRT; return (per-core results, exec_time_ns)."""
    import glob
    import os
    import tempfile

    from concourse import bass2jax

    hook = _get_ntff_hook() if profile else None
    if hook is None:
        return bass2jax.run_bass_via_pjrt(nc, in_maps, n_cores=N_CORES), None, None

    tmpdir = tempfile.mkdtemp(prefix="bass_ntff_")
    try:
        with hook(tmpdir, [0]):
            results = bass2jax.run_bass_via_pjrt(nc, in_maps, n_cores=N_CORES)
    except Exception as e:
        print(f"[kernel] NTFF hook failed ({type(e).__name__}: {e}); plain run")
        return bass2jax.run_bass_via_pjrt(nc, in_maps, n_cores=N_CORES), None, None

    exec_ns = None
    try:
        if glob.glob(os.path.join(tmpdir, "*_body*.ntff")):
            import gauge.profiler
            from concourse._compat import FishPath

            profile_obj = gauge.profiler.Profile(
                profile_path=FishPath(tmpdir),
                kernel_dev_mode=True,
                profile_on_exit=False,
                bass_kernel=nc.m,
                offline_processing=True,
                fname="*_body*",
            )
            prs = profile_obj.to_perfetto(model_index=(0,))
            if prs:
                exec_ns = max(p.exec_time_ns for p in prs if p.exec_time_ns)
    except Exception as e:
        print(f"[kernel] NTFF parse failed ({type(e).__name__}: {e})")
    return results, exec_ns, tmpdir


def kernel(**inputs):
    global LAST_EXEC_NS, LAST_TRACE_DIR
    if "nc" not in _CACHE:
        nc = _build_nc()
        if not nc.is_finalized():
            nc.finalize()
        _CACHE["nc"] = nc
    nc = _CACHE["nc"]

    in_maps = _host_prep(**inputs)
    results, exec_ns, trace_dir = _run(nc, in_maps)
    LAST_EXEC_NS = exec_ns
    LAST_TRACE_DIR = trace_dir
    out = np.concatenate([results[c]["out"] for c in range(N_CORES)], axis=0)
    return out.astype(np.float32)
